# revision 1
# baseline (speedup 1.0000x reference)
"""Multi-head attention (B=4, N=2048, C=1024, H=16) on 8 TRN2 NeuronCores.

Sharding: core = 2*b + half handles batch b, heads half*8 .. half*8+7.
Each core computes QKV for its 8 heads, full attention for them, and a
partial projection (its 512 rows of W_proj). Host sums the two partials
per batch and adds the bias.

All matmul operands are fp16 (1 cycle/row on the PE vs 2 for f32r, same
~11-bit mantissa); accumulation stays fp32 in PSUM. The host pre-casts
weights/x to fp16 and pre-transposes x so x^T tiles DMA in contiguously.

On-chip layout is "transposed": Q^T/K^T [d, n] come straight out of the
QKV matmuls (lhsT = W chunk, rhs = x^T), scores are computed as
S^T[m, n] so that exp(S^T) = P^T is directly the moving operand of the
AV matmul (V chunk stationary). exp is shifted by a constant bias (it
cancels in softmax) to keep P in fp16 range. Row sums of P ride along as
a 65th stationary column of ones; the normalization (reciprocal,
partition broadcast via K=1 matmul, multiply) happens lazily in SBUF so
PSUM banks recycle immediately.
"""

import functools
from contextlib import ExitStack

import numpy as np

import concourse.bass as bass
import concourse.tile as tile
from concourse import bacc, mybir
from concourse.bass_utils import run_bass_kernel_spmd

F32 = mybir.dt.float32
F32R = mybir.dt.float32r
F16 = mybir.dt.float16
AF = mybir.ActivationFunctionType

B, N, C = 4, 2048, 1024
H, D = 16, 64
P = 128
NCORES = 8
HPC = 8            # heads per core
PAIRS = HPC // 2   # 4
DCORE = HPC * D    # 512 attention columns per core
SCALE = float(H) ** -0.5  # 0.25 (faithful to reference: num_heads**-0.5)
EXP_BIAS = -5.0    # exp(scale*s + bias): cancels in softmax, keeps fp16 range
NB = N // 512      # 4 n blocks
NT = N // P        # 16 tiles of 128
CT = C // P        # 8 contraction chunks
VW = D + 1         # V columns per head incl. the ones column (row sums)
MBLK = HPC * VW    # 520 v_sb columns per m-tile

LAST_RESULT = None  # BassKernelResults of the most recent run (for test.py)


def _kernel_body(tc, out_d, xt_d, wq_d, wk_d, wv_d, wp_d):
    nc = tc.nc
    with ExitStack() as ctx:
        const = ctx.enter_context(tc.tile_pool(name="const", bufs=1))
        ones_f = const.tile([P, P], F32)
        nc.vector.memset(ones_f, 1.0)
        ones_bc = const.tile([P, 64], F16)
        nc.vector.tensor_copy(ones_bc, ones_f[:, 0:64])
        ebias = const.tile([P, 1], F32)
        nc.vector.memset(ebias, EXP_BIAS)

        # attT: pair p occupies cols [p*N, (p+1)*N); partitions = 2 heads x 64
        attT_pool = ctx.enter_context(tc.tile_pool(name="attT", bufs=1))
        attT = attT_pool.tile([P, PAIRS * N], F16)

        # PSUM: mm 2 + s 2x2 + av 2 (tags avA/avB) = 8 banks
        ps_mm = ctx.enter_context(tc.tile_pool(name="ps_mm", bufs=2, space="PSUM"))
        ps_s = ctx.enter_context(tc.tile_pool(name="ps_s", bufs=2, space="PSUM"))
        ps_av = ctx.enter_context(tc.tile_pool(name="ps_av", bufs=1, space="PSUM"))

        with ExitStack() as mid:
            # x^T: c-chunk j at cols [j*N, (j+1)*N); DMA'd directly (host
            # pre-transposed x)
            xt_pool = mid.enter_context(tc.tile_pool(name="xt", bufs=1))
            xt = xt_pool.tile([P, CT * N], F16)
            for j in range(CT):
                nc.sync.dma_start(out=xt[:, j * N:(j + 1) * N],
                                  in_=xt_d[j * P:(j + 1) * P, :])
            # V: m-tile m at cols [m*MBLK, ...); head hl at
            # [m*MBLK + hl*VW, +D], then a ones column (for row sums)
            v_pool = mid.enter_context(tc.tile_pool(name="v", bufs=1))
            v_sb = v_pool.tile([P, NT * MBLK], F16)
            ones_cols = v_sb.rearrange("q (g k) -> q g k", k=VW)[:, :, D:VW]
            nc.vector.tensor_copy(
                ones_cols, ones_f.rearrange("q (g k) -> q g k", k=1))

            # ---- Phase B1: V for all 8 heads ----
            with tc.tile_pool(name="wv", bufs=1) as wv_pool:
                wv_sb = wv_pool.tile([P, CT * DCORE], F16)
                for cc in range(CT):
                    nc.sync.dma_start(
                        out=wv_sb[:, cc * DCORE:(cc + 1) * DCORE],
                        in_=wv_d[cc * P:(cc + 1) * P, :])
                for m in range(NT):
                    psv = ps_mm.tile([P, DCORE], F32, tag="mm")
                    for cc in range(CT):
                        nc.tensor.matmul(
                            psv,
                            xt[:, cc * N + m * P: cc * N + (m + 1) * P],
                            wv_sb[:, cc * DCORE:(cc + 1) * DCORE],
                            start=(cc == 0), stop=(cc == CT - 1))
                    nc.vector.tensor_copy(
                        v_sb[:, m * MBLK:(m + 1) * MBLK].rearrange(
                            "q (h k) -> q h k", k=VW)[:, :, 0:D],
                        psv.rearrange("q (h k) -> q h k", k=D))

            # ---- Phases B2 + C, interleaved per head pair ----
            qt_pool = mid.enter_context(tc.tile_pool(name="qt", bufs=2))
            kt_pool = mid.enter_context(tc.tile_pool(name="kt", bufs=2))
            wqk_pool = mid.enter_context(tc.tile_pool(name="wqk", bufs=3))
            pt_pool = mid.enter_context(tc.tile_pool(name="pt", bufs=4))
            rb_pool = mid.enter_context(tc.tile_pool(name="rb", bufs=3))
            rc_pool = mid.enter_context(tc.tile_pool(name="rc", bufs=3))

            for p in range(PAIRS):
                # B2: Q^T and K^T for the pair (partitions: head 2p dims
                # 0-63, head 2p+1 dims 64-127)
                qt = qt_pool.tile([P, N], F16, tag="qt")
                kt = kt_pool.tile([P, N], F16, tag="kt")
                for w_d, dst in ((wq_d, qt), (wk_d, kt)):
                    wt = wqk_pool.tile([P, CT * P], F16, tag="w")
                    nc.sync.dma_start(
                        out=wt.rearrange("q (cc f) -> q cc f", cc=CT),
                        in_=w_d[:, p * P:(p + 1) * P].rearrange(
                            "(cc q) f -> q cc f", q=P))
                    for nb in range(NB):
                        psq = ps_mm.tile([P, 512], F32, tag="mm")
                        for cc in range(CT):
                            nc.tensor.matmul(
                                psq,
                                wt[:, cc * P:(cc + 1) * P],
                                xt[:, cc * N + nb * 512: cc * N + nb * 512 + 512],
                                start=(cc == 0), stop=(cc == CT - 1))
                        nc.vector.tensor_copy(dst[:, nb * 512:(nb + 1) * 512], psq)

                # C: attention for the pair
                for nb in range(NB):
                    nsl = slice(nb * 512, nb * 512 + 512)
                    osl = slice(p * N + nb * 512, p * N + nb * 512 + 512)
                    ps_av_a = ps_av.tile([P, 512], F32, tag="avA")
                    ps_av_b = ps_av.tile([P, 512], F32, tag="avB")
                    for m in range(NT):
                        first = (m == 0)
                        last = (m == NT - 1)
                        ps_s_t = ps_s.tile([P, 1024], F32, tag="s")
                        # scores^T chunk [m-tile, n-block]; heads row-tiled
                        nc.tensor.matmul(
                            ps_s_t[:, 0:512],
                            kt[0:64, m * P:(m + 1) * P],
                            qt[0:64, nsl],
                            start=True, stop=True)
                        nc.tensor.matmul(
                            ps_s_t[:, 512:1024],
                            kt[64:128, m * P:(m + 1) * P],
                            qt[64:128, nsl],
                            start=True, stop=True)
                        pt = pt_pool.tile([P, 1024], F16, tag="pt")
                        nc.scalar.activation(pt, ps_s_t, AF.Exp,
                                             scale=SCALE, bias=ebias)
                        # AV with fused row-sums: lhsT = [V_h | 1] (M = 65);
                        # partition 64 accumulates the softmax denominators
                        vbase = m * MBLK + 2 * p * VW
                        nc.tensor.matmul(
                            ps_av_a[0:VW, :],
                            v_sb[:, vbase: vbase + VW],
                            pt[:, 0:512],
                            start=first, stop=last, skip_group_check=True)
                        nc.tensor.matmul(
                            ps_av_b[0:VW, :],
                            v_sb[:, vbase + VW: vbase + 2 * VW],
                            pt[:, 512:1024],
                            start=first, stop=last, skip_group_check=True)
                    # evict eagerly (free the PSUM banks), normalize lazily
                    rc = rc_pool.tile([P, 1024], F16, tag="rc")
                    nc.vector.tensor_copy(rc[64:65, 0:512], ps_av_a[D:VW, :])
                    nc.vector.tensor_copy(rc[64:65, 512:1024], ps_av_b[D:VW, :])
                    nc.vector.tensor_copy(attT[0:64, osl], ps_av_a[0:64, :])
                    tmb = rb_pool.tile([64, 512], F16, tag="tmb")
                    nc.vector.tensor_copy(tmb, ps_av_b[0:64, :])
                    # lazy: fp16 reciprocal + K=1 broadcast matmul + mul
                    with nc.allow_low_precision(
                            reason="softmax recip rounding is benign"):
                        nc.vector.reciprocal(rc[64:65, :], rc[64:65, :])
                    ps_rb_a = ps_mm.tile([P, 512], F32, tag="mm")
                    nc.tensor.matmul(
                        ps_rb_a[0:64, :], ones_bc[64:65, :],
                        rc[64:65, 0:512],
                        start=True, stop=True, tile_position=(64, 0),
                        skip_group_check=True)
                    ps_rb_b = ps_mm.tile([P, 512], F32, tag="mm")
                    nc.tensor.matmul(
                        ps_rb_b[0:64, :], ones_bc[64:65, :],
                        rc[64:65, 512:1024],
                        start=True, stop=True, tile_position=(64, 0),
                        skip_group_check=True)
                    rb = rb_pool.tile([64, 1024], F32, tag="rb")
                    nc.vector.tensor_copy(rb[:, 0:512], ps_rb_a[0:64, :])
                    nc.vector.tensor_copy(rb[:, 512:1024], ps_rb_b[0:64, :])
                    nc.vector.tensor_mul(attT[0:64, osl],
                                         attT[0:64, osl], rb[:, 0:512])
                    nc.vector.tensor_mul(tmb, tmb, rb[:, 512:1024])
                    # head B's rows sit at partitions 0-63; shift to 64-127
                    nc.sync.dma_start(out=attT[64:128, osl], in_=tmb)

        # ---- Phase D: partial projection out = attT.T @ wp ----
        with tc.tile_pool(name="wp", bufs=1) as wp_pool, \
                tc.tile_pool(name="stage", bufs=3) as stage_pool:
            wp_sb = wp_pool.tile([P, PAIRS * C], F16)
            for dc in range(PAIRS):
                nc.sync.dma_start(out=wp_sb[:, dc * C:(dc + 1) * C],
                                  in_=wp_d[dc * P:(dc + 1) * P, :])
            for i in range(NT):
                for co in range(2):
                    psp = ps_mm.tile([P, 512], F32, tag="mm")
                    for dc in range(PAIRS):
                        nc.tensor.matmul(
                            psp,
                            attT[:, dc * N + i * P: dc * N + (i + 1) * P],
                            wp_sb[:, dc * C + co * 512: dc * C + co * 512 + 512],
                            start=(dc == 0), stop=(dc == PAIRS - 1))
                    st = stage_pool.tile([P, 512], F32, tag="st")
                    nc.vector.tensor_copy(st, psp)
                    nc.sync.dma_start(
                        out=out_d[i * P:(i + 1) * P, co * 512: co * 512 + 512],
                        in_=st)


@functools.lru_cache(maxsize=1)
def build_nc():
    nc = bacc.Bacc("TRN2", target_bir_lowering=False, debug=False)
    xt_d = nc.dram_tensor("xt_local", [C, N], F16, kind="ExternalInput").ap()
    wq_d = nc.dram_tensor("wq", [C, DCORE], F16, kind="ExternalInput").ap()
    wk_d = nc.dram_tensor("wk", [C, DCORE], F16, kind="ExternalInput").ap()
    wv_d = nc.dram_tensor("wv", [C, DCORE], F16, kind="ExternalInput").ap()
    wp_d = nc.dram_tensor("wp", [DCORE, C], F16, kind="ExternalInput").ap()
    out_d = nc.dram_tensor("out_partial", [N, C], F32, kind="ExternalOutput").ap()
    with tile.TileContext(nc) as tc:
        _kernel_body(tc, out_d, xt_d, wq_d, wk_d, wv_d, wp_d)
    nc.compile()
    return nc


def make_in_maps(x, W_qkv, W_proj):
    in_maps = []
    for core in range(NCORES):
        b, half = core // 2, core % 2
        h0 = half * HPC
        in_maps.append({
            "xt_local": np.ascontiguousarray(x[b].T.astype(np.float16)),
            "wq": np.ascontiguousarray(
                W_qkv[:, 0 * C + h0 * D: 0 * C + h0 * D + DCORE].astype(np.float16)),
            "wk": np.ascontiguousarray(
                W_qkv[:, 1 * C + h0 * D: 1 * C + h0 * D + DCORE].astype(np.float16)),
            "wv": np.ascontiguousarray(
                W_qkv[:, 2 * C + h0 * D: 2 * C + h0 * D + DCORE].astype(np.float16)),
            "wp": np.ascontiguousarray(
                W_proj[h0 * D: h0 * D + DCORE, :].astype(np.float16)),
        })
    return in_maps


def kernel(x, W_qkv, W_proj, b_proj, trace=False):
    x = np.asarray(x, dtype=np.float32)
    W_qkv = np.asarray(W_qkv, dtype=np.float32)
    W_proj = np.asarray(W_proj, dtype=np.float32)
    b_proj = np.asarray(b_proj, dtype=np.float32)

    nc = build_nc()
    in_maps = make_in_maps(x, W_qkv, W_proj)

    global LAST_RESULT
    res = run_bass_kernel_spmd(nc, in_maps, list(range(NCORES)), trace=trace)
    LAST_RESULT = res

    out = np.empty((B, N, C), dtype=np.float32)
    for b in range(B):
        out[b] = (res.results[2 * b]["out_partial"]
                  + res.results[2 * b + 1]["out_partial"]
                  + b_proj[None, :])
    return out

